# revision 3
# baseline (speedup 1.0000x reference)
"""Trainium2 Bass kernel for NonLocalAttention (fused 1x1 convs + spatial softmax attention).

Reference computation (N=2, C=64, FC=64, CR=32, H=W=96, HW=9216):
    q = relu(wq @ x + bq)          [N, 32, HW]
    k = relu(wk @ fm + bk)         [N, 32, HW]
    v = relu(wa @ fm + ba)         [N, 64, HW]
    s = softmax(q^T k, axis=keys)  [N, HW, HW]
    o = s @ v^T                    [N, HW, 64]
    out = relu(wo @ [x; o^T] + bo) [N, 64, HW]

Sharding: 8 cores = batch(2) x query-rows(4).  Each core handles 2304 query
pixels of one batch element and needs the full fusionmap of that batch.

Per-core kernel (flash-style, score never goes to HBM):
  - score is computed TRANSPOSED: st[key, q] = k^T q via row-packed (K=32)
    matmuls, 3 key-tiles of 128 at a time into 3 PSUM banks.
  - exp on ScalarE directly PSUM->SBUF over [128, 3*qn] (no max subtraction:
    scores are bounded, max ~6.5).
  - second matmul contracts keys with lhsT = [v^T | 1] so PSUM row 64
    accumulates the softmax denominator for free.
  - normalize with reciprocal + DMA partition-broadcast, then the output
    1x1 conv (wo) is fused in, relu, DMA out.
"""

import sys

sys.path.insert(0, "/opt/trn_rl_repo")

from contextlib import ExitStack

import numpy as np

import concourse.bacc as bacc
import concourse.bass as bass
import concourse.tile as tile
from concourse import mybir
from concourse import bass_utils

C = 64
FC = 64
CR = 32
N = 2
H = W = 96
HW = H * W            # 9216
NCORES = 8
QPC = HW // 4         # queries per core = 2304
NKT = HW // 128       # 72 key tiles
G = 3                 # row-packing group (3 key tiles concurrently)
NJ = NKT // G         # 24 key-tile groups
QCHUNKS = [(0, 512), (512, 512), (1024, 512), (1536, 512), (2048, 256)]

F32 = mybir.dt.float32
F32R = mybir.dt.float32r

# set False to run all matmuls in plain fp32 (4x slower PE, bit-accurate)
USE_F32R = False


def _mmdt(ap):
    return ap.bitcast(F32R) if USE_F32R else ap


def build_bass():
    nc = bacc.Bacc(
        "TRN2", target_bir_lowering=False, debug=False, num_devices=NCORES
    )

    x_aug = nc.dram_tensor("x_aug", [C + 1, QPC], F32, kind="ExternalInput")
    fm_aug = nc.dram_tensor("fm_aug", [FC + 1, HW], F32, kind="ExternalInput")
    wq_aug = nc.dram_tensor("wq_aug", [C + 1, CR], F32, kind="ExternalInput")
    wk_aug = nc.dram_tensor("wk_aug", [FC + 1, CR], F32, kind="ExternalInput")
    wa_aug = nc.dram_tensor("wa_aug", [FC + 1, C], F32, kind="ExternalInput")
    wox_aug = nc.dram_tensor("wox_aug", [C + 1, C], F32, kind="ExternalInput")
    woa_t = nc.dram_tensor("woa_t", [C, C], F32, kind="ExternalInput")
    out_d = nc.dram_tensor("out_c", [C, QPC], F32, kind="ExternalOutput")

    with tile.TileContext(nc) as tc, ExitStack() as ctx:
        consts = ctx.enter_context(tc.tile_pool(name="consts", bufs=1))
        stp = ctx.enter_context(tc.tile_pool(name="stp", bufs=3))
        wk_pool = ctx.enter_context(tc.tile_pool(name="work", bufs=3))
        psA = ctx.enter_context(tc.tile_pool(name="psA", bufs=2, space="PSUM"))
        psO = ctx.enter_context(tc.tile_pool(name="psO", bufs=1, space="PSUM"))
        psF = ctx.enter_context(tc.tile_pool(name="psF", bufs=1, space="PSUM"))

        # ---- constants / inputs in SBUF ----
        FM = consts.tile([FC + 1, HW], F32)         # fusionmap + ones row
        XA = consts.tile([C + 1, QPC], F32)         # x chunk + ones row
        WQ = consts.tile([C + 1, CR], F32)
        WK = consts.tile([FC + 1, CR], F32)
        WA = consts.tile([FC + 1, C], F32)
        WOX = consts.tile([C + 1, C], F32)
        WOA = consts.tile([C, C], F32)
        nc.sync.dma_start(FM[:], fm_aug.ap())
        nc.sync.dma_start(XA[:], x_aug.ap())
        nc.sync.dma_start(WQ[:], wq_aug.ap())
        nc.sync.dma_start(WK[:], wk_aug.ap())
        nc.sync.dma_start(WA[:], wa_aug.ap())
        nc.sync.dma_start(WOX[:], wox_aug.ap())
        nc.sync.dma_start(WOA[:], woa_t.ap())

        # KR: k channels row-packed: partitions 32g..32g+31 hold key tile
        # kt=3j+g at free block j.  QR: q replicated on partition groups 0..2.
        KR = consts.tile([128, NJ, 128], F32)
        QR = consts.tile([128, QPC], F32)
        # VT: [keys(128), kt, C+1]; column C stays 1.0 (denominator trick)
        VT = consts.tile([128, NKT, C + 1], F32)
        nc.vector.memset(VT[:], 1.0)

        # ---- phase 1: q / k / v 1x1 convs ----
        # q: out[32g+c, q] = q[c, q] for g=0..2 via column tiling
        for q0, qn in QCHUNKS:
            ps = psA.tile([128, G, 512], F32, tag="sc")
            for g in range(G):
                nc.tensor.matmul(
                    ps[32 * g : 32 * g + 32, 0, 0:qn],
                    _mmdt(WQ[:]),
                    _mmdt(XA[:, q0 : q0 + qn]),
                )
            nc.vector.tensor_scalar_max(
                QR[0:96, q0 : q0 + qn], ps[0:96, 0, 0:qn], 0.0
            )

        # k: interleaved into KR.  For each round of 4 j-groups fill one bank.
        for r in range(NJ // 4):
            ps = psA.tile([128, G, 512], F32, tag="sc")
            for jj in range(4):
                j = 4 * r + jj
                for g in range(G):
                    kt = G * j + g
                    nc.tensor.matmul(
                        ps[32 * g : 32 * g + 32, 0, 128 * jj : 128 * (jj + 1)],
                        _mmdt(WK[:]),
                        _mmdt(FM[:, 128 * kt : 128 * (kt + 1)]),
                    )
            nc.vector.tensor_scalar_max(
                KR[0:96, 4 * r : 4 * r + 4, :], ps[0:96, 0, 0:512], 0.0
            )

        # v^T: out[key, c] per key tile; 8 key tiles per PSUM bank
        for r in range(NKT // 8):
            ps = psA.tile([128, G, 512], F32, tag="sc")
            for i in range(8):
                kt = 8 * r + i
                nc.tensor.matmul(
                    ps[:, 0, 64 * i : 64 * (i + 1)],
                    _mmdt(FM[:, 128 * kt : 128 * (kt + 1)]),
                    _mmdt(WA[:]),
                )
            nc.vector.tensor_scalar_max(
                VT[:, 8 * r : 8 * r + 8, 0:C], ps[:, 0, 0:512], 0.0
            )

        # ---- phase 2: attention + output conv, per query chunk ----
        for q0, qn in QCHUNKS:
            acc = psO.tile([128, 512], F32, tag="acc")
            for j in range(NJ):
                sc = psA.tile([128, G, 512], F32, tag="sc")
                for g in range(G):
                    nc.tensor.matmul(
                        sc[:, g, 0:qn],
                        _mmdt(KR[32 * g : 32 * g + 32, j, :]),
                        _mmdt(QR[32 * g : 32 * g + 32, q0 : q0 + qn]),
                    )
                st = stp.tile([128, G, 512], F32, tag="st")
                nc.scalar.activation(
                    st[:, :, 0:qn],
                    sc[:, :, 0:qn],
                    mybir.ActivationFunctionType.Exp,
                )
                for g in range(G):
                    kt = G * j + g
                    nc.tensor.matmul(
                        acc[0 : C + 1, 0:qn],
                        _mmdt(VT[:, kt, :]),
                        _mmdt(st[:, g, 0:qn]),
                        start=(j == 0 and g == 0),
                        stop=(j == NJ - 1 and g == G - 1),
                    )

            # normalize: row C of acc = sum_k exp(score)
            recip = wk_pool.tile([1, 512], F32, tag="recip")
            nc.vector.reciprocal(recip[:, 0:qn], acc[C : C + 1, 0:qn])
            rb = wk_pool.tile([C, 512], F32, tag="rb")
            nc.gpsimd.partition_broadcast(rb[:, 0:qn], recip[0:1, 0:qn])
            attn = wk_pool.tile([C, 512], F32, tag="attn")
            nc.vector.tensor_mul(attn[:, 0:qn], acc[0:C, 0:qn], rb[:, 0:qn])

            # out = relu(wo_x @ x + wo_a @ attn + bo)
            fin = psF.tile([128, 512], F32, tag="fin")
            nc.tensor.matmul(
                fin[0:C, 0:qn],
                _mmdt(WOX[:]),
                _mmdt(XA[:, q0 : q0 + qn]),
                start=True,
                stop=False,
            )
            nc.tensor.matmul(
                fin[0:C, 0:qn],
                _mmdt(WOA[:]),
                _mmdt(attn[:, 0:qn]),
                start=False,
                stop=True,
            )
            outs = wk_pool.tile([C, 512], F32, tag="outs")
            nc.vector.tensor_scalar_max(outs[:, 0:qn], fin[0:C, 0:qn], 0.0)
            nc.sync.dma_start(out_d.ap()[:, q0 : q0 + qn], outs[:, 0:qn])

    nc.compile()
    return nc


_NC_CACHE = None


def _get_nc():
    global _NC_CACHE
    if _NC_CACHE is None:
        _NC_CACHE = build_bass()
    return _NC_CACHE


def make_in_maps(x, fusionmap, wq, bq, wk, bk, wa, ba, wo, bo):
    x = np.asarray(x, np.float32)
    fm = np.asarray(fusionmap, np.float32)
    xf = x.reshape(N, C, HW)
    fmf = fm.reshape(N, FC, HW)
    ones_hw = np.ones((1, HW), np.float32)
    wq_aug = np.concatenate(
        [np.asarray(wq).T, np.asarray(bq)[None, :]], 0
    ).astype(np.float32)
    wk_aug = np.concatenate(
        [np.asarray(wk).T, np.asarray(bk)[None, :]], 0
    ).astype(np.float32)
    wa_aug = np.concatenate(
        [np.asarray(wa).T, np.asarray(ba)[None, :]], 0
    ).astype(np.float32)
    wo = np.asarray(wo, np.float32)
    wox_aug = np.concatenate(
        [wo[:, :C].T, np.asarray(bo)[None, :]], 0
    ).astype(np.float32)
    woa_t = np.ascontiguousarray(wo[:, C:].T).astype(np.float32)

    in_maps = []
    for core in range(NCORES):
        n, c = divmod(core, 4)
        x_chunk = xf[n][:, c * QPC : (c + 1) * QPC]
        x_aug = np.concatenate([x_chunk, ones_hw[:, :QPC]], 0)
        fm_aug = np.concatenate([fmf[n], ones_hw], 0)
        in_maps.append(
            {
                "x_aug": np.ascontiguousarray(x_aug),
                "fm_aug": np.ascontiguousarray(fm_aug),
                "wq_aug": wq_aug,
                "wk_aug": wk_aug,
                "wa_aug": wa_aug,
                "wox_aug": wox_aug,
                "woa_t": woa_t,
            }
        )
    return in_maps


def run(in_maps, trace=False, tmpdir=None):
    nc = _get_nc()
    return bass_utils.run_bass_kernel_spmd(
        nc,
        in_maps,
        core_ids=list(range(NCORES)),
        trace=trace,
        tmpdir=tmpdir,
    )


def kernel(**inputs):
    in_maps = make_in_maps(**inputs)
    res = run(in_maps)
    out = np.empty((N, C, HW), np.float32)
    for core in range(NCORES):
        n, c = divmod(core, 4)
        out[n][:, c * QPC : (c + 1) * QPC] = res.results[core]["out_c"]
    return out.reshape(N, C, H, W)


if __name__ == "__main__":
    import reference

    inputs = {k: np.asarray(v) for k, v in reference.setup_inputs().items()}
    got = kernel(**inputs)
    print("kernel output", got.shape, got.dtype)
